# revision 6
# baseline (speedup 1.0000x reference)
import numpy as np

import concourse.bass as bass
import concourse.mybir as mybir
import concourse.tile as tile
from concourse import bacc
from concourse.bass_utils import run_bass_kernel_spmd
from concourse.masks import make_identity

AF = mybir.ActivationFunctionType
ALU = mybir.AluOpType
F32 = mybir.dt.float32
BF16 = mybir.dt.bfloat16

BSZ, TGT, SRC, HSZ = 4, 256, 1024, 256
TSH = TGT // 2
NC = 8

W0 = 0.576
D_LIN = 0.1833282930396873
BETAS = [
    0.557414326754684, 0.19372747439423482, 0.07667426380524862,
    0.03092129827665697, 0.012485548131372468, 0.005051991717181822,
    0.0020242424061486164, 0.000858500828339962, 0.0002858874721107346,
    0.00020297013401236646,
]
NH = len(BETAS)
HALFPI = float(np.pi / 2)

_CHAIN_ENGINES = ("vector", "gpsimd")


def _build_nc():
    nc = bacc.Bacc(None, target_bir_lowering=False)

    query_s = nc.declare_dram_parameter("query_s", [TSH, HSZ], F32, isOutput=False)
    key_s = nc.declare_dram_parameter("key_s", [SRC, HSZ], F32, isOutput=False)
    wq = nc.declare_dram_parameter("wq", [HSZ, HSZ], F32, isOutput=False)
    wk = nc.declare_dram_parameter("wk", [HSZ, HSZ], F32, isOutput=False)
    vv = nc.declare_dram_parameter("vv", [HSZ], F32, isOutput=False)
    out = nc.declare_dram_parameter("out", [TSH, SRC], F32, isOutput=True)

    with tile.TileContext(nc) as tc:
        with (
            tc.tile_pool(name="consts", bufs=1) as consts,
            tc.tile_pool(name="sb", bufs=1) as sb,
            tc.tile_pool(name="tmp", bufs=3) as tmp,
            tc.tile_pool(name="psA", bufs=3, space=bass.MemorySpace.PSUM) as psA,
            tc.tile_pool(name="psB", bufs=1, space=bass.MemorySpace.PSUM) as psB,
            tc.tile_pool(name="psC", bufs=1, space=bass.MemorySpace.PSUM) as psC,
        ):
            ident = consts.tile([128, 128], F32)
            make_identity(nc, ident[:])
            halfpi = consts.tile([128, 1], F32)
            nc.vector.memset(halfpi[:], HALFPI)
            zero = consts.tile([128, 1], F32)
            nc.vector.memset(zero[:], 0.0)

            qsb = sb.tile([128, 2, 128], F32)
            nc.sync.dma_start(qsb[:], query_s.rearrange("t (b h) -> t b h", h=128))
            wq2 = sb.tile([128, 2, HSZ], F32)
            nc.scalar.dma_start(wq2[:], wq.rearrange("(a p) h -> p a h", p=128))
            wk2 = sb.tile([128, 2, HSZ], F32)
            nc.gpsimd.dma_start(wk2[:], wk.rearrange("(a p) h -> p a h", p=128))
            vcol = consts.tile([128, 2], F32)
            nc.gpsimd.dma_start(vcol[:], vv.rearrange("(a p) -> p a", p=128))
            ksb = []
            for i in range(4):
                kt = sb.tile([128, 2, HSZ], F32, tag=f"ksb{i}", name=f"ksb{i}")
                eng = nc.sync if i % 2 == 0 else nc.scalar
                eng.dma_start(
                    kt[:],
                    key_s.rearrange("(c p2 p) h -> c p p2 h", c=4, p2=2)[i],
                )
                ksb.append(kt)

            cv = consts.tile([128, 2, 2 * NH + 1], F32)
            for oh in range(2):
                nc.vector.tensor_scalar(
                    cv[:, oh, 0:1], vcol[:, oh : oh + 1], float(D_LIN), None, ALU.mult)
                for n in range(NH):
                    b = float(BETAS[n])
                    nc.vector.tensor_scalar(
                        cv[:, oh, 1 + n : 2 + n], vcol[:, oh : oh + 1], b, None, ALU.mult)
                    nc.vector.tensor_scalar(
                        cv[:, oh, 1 + NH + n : 2 + NH + n], vcol[:, oh : oh + 1], b, None,
                        ALU.mult)

            qT = sb.tile([128, 2, TSH], F32)
            for hh in range(2):
                pt = psA.tile([128, 128], F32, tag="tp")
                nc.tensor.transpose(pt[:], qsb[:, hh, :], ident[:])
                nc.scalar.copy(qT[:, hh, :], pt[:])
            wqT = sb.tile([128, 2, HSZ], F32)
            for oh in range(2):
                for hh in range(2):
                    pt = psA.tile([128, 128], F32, tag="tp")
                    nc.tensor.transpose(pt[:], wq2[:, oh, hh * 128 : (hh + 1) * 128], ident[:])
                    nc.scalar.copy(wqT[:, hh, oh * 128 : (oh + 1) * 128], pt[:])
            Sq = [sb.tile([128, 2, TSH], BF16, tag=f"Sq{n}", name=f"Sq{n}") for n in range(NH)]
            Cq = [sb.tile([128, 2, TSH], BF16, tag=f"Cq{n}", name=f"Cq{n}") for n in range(NH)]
            for oh in range(2):
                pq = psA.tile([128, TSH], F32, tag="tp")
                for hh in range(2):
                    nc.tensor.matmul(
                        pq[:], wqT[:, hh, oh * 128 : (oh + 1) * 128], qT[:, hh, :],
                        start=(hh == 0), stop=(hh == 1))
                nc.scalar.activation(Sq[0][:, oh, :], pq[:], AF.Sin, bias=zero[:], scale=W0)
                nc.scalar.activation(Cq[0][:, oh, :], pq[:], AF.Sin, bias=halfpi[:], scale=W0)

            Dq = sb.tile([128, 2, TSH], BF16)
            nc.vector.tensor_scalar(Dq[:], Cq[0][:], 2.0, None, ALU.mult)
            for n in range(1, NH):
                if n == 1:
                    nc.vector.tensor_tensor(Sq[1][:], Dq[:], Sq[0][:], ALU.mult)
                    t0 = tmp.tile([128, 2, TSH], BF16, tag="qc")
                    nc.vector.tensor_tensor(t0[:], Dq[:], Cq[0][:], ALU.mult)
                    nc.vector.tensor_scalar(Cq[1][:], t0[:], 1.0, None, ALU.subtract)
                else:
                    t1 = tmp.tile([128, 2, TSH], BF16, tag="qc")
                    nc.vector.tensor_tensor(t1[:], Dq[:], Sq[n - 1][:], ALU.mult)
                    nc.vector.tensor_tensor(Sq[n][:], t1[:], Sq[n - 2][:], ALU.subtract)
                    t2 = tmp.tile([128, 2, TSH], BF16, tag="qc")
                    nc.vector.tensor_tensor(t2[:], Dq[:], Cq[n - 1][:], ALU.mult)
                    nc.vector.tensor_tensor(Cq[n][:], t2[:], Cq[n - 2][:], ALU.subtract)

            ones_b = consts.tile([128, 2, TSH], BF16)
            nc.vector.memset(ones_b[:], 1.0)
            lh_d = sb.tile([128, 2, TSH], BF16)
            lh_S = [sb.tile([128, 2, TSH], BF16, tag=f"lhS{n}", name=f"lhS{n}") for n in range(NH)]
            lh_C = [sb.tile([128, 2, TSH], BF16, tag=f"lhC{n}", name=f"lhC{n}") for n in range(NH)]
            for oh in range(2):
                nc.vector.tensor_scalar(
                    lh_d[:, oh, :], ones_b[:, oh, :], cv[:, oh, 0:1], None, ALU.mult)
                for n in range(NH):
                    nc.vector.tensor_scalar(
                        lh_S[n][:, oh, :], Sq[n][:, oh, :], cv[:, oh, 1 + n : 2 + n],
                        None, ALU.mult)
                    nc.vector.tensor_scalar(
                        lh_C[n][:, oh, :], Cq[n][:, oh, :],
                        cv[:, oh, 1 + NH + n : 2 + NH + n], None, ALU.mult)

            wkT = sb.tile([128, 2, HSZ], F32)
            for oh in range(2):
                for hh in range(2):
                    pt = psA.tile([128, 128], F32, tag="tp")
                    nc.tensor.transpose(pt[:], wk2[:, oh, hh * 128 : (hh + 1) * 128], ident[:])
                    nc.scalar.copy(wkT[:, hh, oh * 128 : (oh + 1) * 128], pt[:])
            kT = sb.tile([128, 2, SRC], F32)
            for blk in range(8):
                src_tile = ksb[blk // 2][:, blk % 2, :]
                for hh in range(2):
                    pt = psA.tile([128, 128], F32, tag="tp")
                    nc.tensor.transpose(pt[:], src_tile[:, hh * 128 : (hh + 1) * 128], ident[:])
                    eng = nc.scalar if (blk + hh) % 2 == 0 else nc.vector
                    eng_copy = (
                        nc.scalar.copy if (blk + hh) % 2 == 0
                        else lambda o, i: nc.vector.tensor_copy(o, i)
                    )
                    eng_copy(kT[:, hh, blk * 128 : (blk + 1) * 128], pt[:])

            Sk = [sb.tile([128, 2, SRC], BF16, tag=f"Sk{n}", name=f"Sk{n}") for n in range(NH)]
            Ck = [sb.tile([128, 2, SRC], BF16, tag=f"Ck{n}", name=f"Ck{n}") for n in range(NH)]
            kraw = sb.tile([128, 2, SRC], BF16)
            for oh in range(2):
                pk = psB.tile([128, SRC], F32, tag="pk", name=f"pk{oh}")
                for sc in range(2):
                    for hh in range(2):
                        nc.tensor.matmul(
                            pk[:, sc * 512 : (sc + 1) * 512],
                            wkT[:, hh, oh * 128 : (oh + 1) * 128],
                            kT[:, hh, sc * 512 : (sc + 1) * 512],
                            start=(hh == 0), stop=(hh == 1))
                nc.scalar.activation(Sk[0][:, oh, :], pk[:], AF.Sin, bias=zero[:], scale=W0)
                nc.scalar.activation(Ck[0][:, oh, :], pk[:], AF.Sin, bias=halfpi[:], scale=W0)
                nc.scalar.copy(kraw[:, oh, :], pk[:])

            Dk = sb.tile([128, 2, SRC], BF16)
            nc.vector.tensor_scalar(Dk[:], Ck[0][:], 2.0, None, ALU.mult)
            eng_i = 0

            def chain_engine():
                nonlocal eng_i
                e = nc.vector if eng_i % 3 != 2 else nc.gpsimd
                eng_i += 1
                return e

            for n in range(1, NH):
                if n == 1:
                    chain_engine().tensor_tensor(Sk[1][:], Dk[:], Sk[0][:], ALU.mult)
                    t0 = tmp.tile([128, 2, SRC], BF16, tag="kc")
                    chain_engine().tensor_tensor(t0[:], Dk[:], Ck[0][:], ALU.mult)
                    chain_engine().tensor_scalar(Ck[1][:], t0[:], 1.0, None, ALU.subtract)
                else:
                    t1 = tmp.tile([128, 2, SRC], BF16, tag="kc")
                    chain_engine().tensor_tensor(t1[:], Dk[:], Sk[n - 1][:], ALU.mult)
                    chain_engine().tensor_tensor(Sk[n][:], t1[:], Sk[n - 2][:], ALU.subtract)
                    t2 = tmp.tile([128, 2, SRC], BF16, tag="kc")
                    chain_engine().tensor_tensor(t2[:], Dk[:], Ck[n - 1][:], ALU.mult)
                    chain_engine().tensor_tensor(Ck[n][:], t2[:], Ck[n - 2][:], ALU.subtract)

            psc = psC.tile([128, SRC], F32)
            mms = [(lh_d, kraw)]
            for n in range(NH):
                mms.append((lh_S[n], Ck[n]))
                mms.append((lh_C[n], Sk[n]))
            for sc in range(2):
                first = True
                for mi, (lh, rh) in enumerate(mms):
                    for oh in range(2):
                        last = (mi == len(mms) - 1) and (oh == 1)
                        nc.tensor.matmul(
                            psc[:, sc * 512 : (sc + 1) * 512],
                            lh[:, oh, :],
                            rh[:, oh, sc * 512 : (sc + 1) * 512],
                            start=first, stop=last)
                        first = False

            negmax = sb.tile([128, 1], F32)
            nc.vector.tensor_reduce(
                negmax[:], psc[:], axis=mybir.AxisListType.X, op=ALU.max, negate=True)
            esb = sb.tile([128, SRC], F32)
            nc.scalar.activation(esb[:], psc[:], AF.Exp, bias=negmax[:])
            denom = sb.tile([128, 1], F32)
            nc.vector.tensor_reduce(
                denom[:], esb[:], axis=mybir.AxisListType.X, op=ALU.add)
            rden = sb.tile([128, 1], F32)
            nc.vector.reciprocal(rden[:], denom[:])
            outsb = sb.tile([128, SRC], F32)
            nc.vector.tensor_scalar(outsb[:], esb[:], rden[:], None, ALU.mult)
            nc.sync.dma_start(out[:], outsb[:])

    nc.compile()
    return nc


_NC_CACHE = None


def kernel(**inputs) -> np.ndarray:
    global _NC_CACHE
    query = np.ascontiguousarray(np.asarray(inputs["query"], dtype=np.float32))
    key = np.ascontiguousarray(np.asarray(inputs["key"], dtype=np.float32))
    Wq = np.ascontiguousarray(np.asarray(inputs["Wq"], dtype=np.float32))
    Wk = np.ascontiguousarray(np.asarray(inputs["Wk"], dtype=np.float32))
    v = np.ascontiguousarray(np.asarray(inputs["v"], dtype=np.float32))

    if _NC_CACHE is None:
        _NC_CACHE = _build_nc()
    nc = _NC_CACHE

    in_maps = []
    for c in range(NC):
        b, th = c // 2, c % 2
        in_maps.append({
            "query_s": query[b, th * TSH : (th + 1) * TSH, :],
            "key_s": key[b],
            "wq": Wq,
            "wk": Wk,
            "vv": v,
        })
    res = run_bass_kernel_spmd(nc, in_maps, core_ids=list(range(NC)))
    out = np.empty((BSZ, TGT, SRC), dtype=np.float32)
    for c in range(NC):
        b, th = c // 2, c % 2
        out[b, th * TSH : (th + 1) * TSH, :] = res.results[c]["out"]
    return out


if __name__ == "__main__":
    rng = np.random.default_rng(0)
    ins = {
        "query": rng.standard_normal((BSZ, TGT, HSZ), dtype=np.float32),
        "key": rng.standard_normal((BSZ, SRC, HSZ), dtype=np.float32),
        "Wq": rng.standard_normal((HSZ, HSZ), dtype=np.float32) / 16,
        "Wk": rng.standard_normal((HSZ, HSZ), dtype=np.float32) / 16,
        "v": rng.standard_normal((HSZ,), dtype=np.float32) / 16,
        "v_bias": np.zeros(1, dtype=np.float32),
    }
    o = kernel(**ins)
    print("out", o.shape, o.dtype, o.sum(-1)[:2, :4])


# revision 7
# speedup vs baseline: 1.8143x; 1.8143x over previous
import numpy as np

import concourse.bass as bass
import concourse.mybir as mybir
import concourse.tile as tile
from concourse import bacc
from concourse.bass_utils import run_bass_kernel_spmd
from concourse.masks import make_identity

AF = mybir.ActivationFunctionType
ALU = mybir.AluOpType
F32 = mybir.dt.float32
BF16 = mybir.dt.bfloat16

BSZ, TGT, SRC, HSZ = 4, 256, 1024, 256
TSH = TGT // 2
NC = 8

W0 = 0.6136
D_LIN = 0.19527216975465894
BETAS = [
    0.5478717234406725, 0.1822984439478343, 0.06833296943058284,
    0.026017365591941354, 0.009861006814128156, 0.003851711284300151,
    0.0013005418391422178, 0.0007527608990423636,
]
NH = len(BETAS)
HALFPI = float(np.pi / 2)


def _build_nc():
    nc = bacc.Bacc(None, target_bir_lowering=False)

    query_s = nc.declare_dram_parameter("query_s", [TSH, HSZ], F32, isOutput=False)
    key_s = nc.declare_dram_parameter("key_s", [SRC, HSZ], F32, isOutput=False)
    wq = nc.declare_dram_parameter("wq", [HSZ, HSZ], F32, isOutput=False)
    wk = nc.declare_dram_parameter("wk", [HSZ, HSZ], F32, isOutput=False)
    vv = nc.declare_dram_parameter("vv", [HSZ], F32, isOutput=False)
    out = nc.declare_dram_parameter("out", [TSH, SRC], F32, isOutput=True)

    with tile.TileContext(nc) as tc:
        with (
            tc.tile_pool(name="consts", bufs=1) as consts,
            tc.tile_pool(name="sb", bufs=1) as sb,
            tc.tile_pool(name="psA", bufs=3, space=bass.MemorySpace.PSUM) as psA,
            tc.tile_pool(name="psB", bufs=1, space=bass.MemorySpace.PSUM) as psB,
            tc.tile_pool(name="psC", bufs=1, space=bass.MemorySpace.PSUM) as psC,
        ):
            ident = consts.tile([128, 128], F32)
            make_identity(nc, ident[:])
            halfpi = consts.tile([128, 1], F32)
            nc.vector.memset(halfpi[:], HALFPI)
            zero = consts.tile([128, 1], F32)
            nc.vector.memset(zero[:], 0.0)

            qsb = sb.tile([128, 2, 128], F32)
            nc.sync.dma_start(qsb[:], query_s.rearrange("t (b h) -> t b h", h=128))
            wq2 = sb.tile([128, 2, HSZ], F32)
            nc.scalar.dma_start(wq2[:], wq.rearrange("(a p) h -> p a h", p=128))
            wk2 = sb.tile([128, 2, HSZ], F32)
            nc.gpsimd.dma_start(wk2[:], wk.rearrange("(a p) h -> p a h", p=128))
            vcol = consts.tile([128, 2], F32)
            nc.gpsimd.dma_start(vcol[:], vv.rearrange("(a p) -> p a", p=128))
            ksb = []
            for i in range(4):
                kt = sb.tile([128, 2, HSZ], F32, tag=f"ksb{i}", name=f"ksb{i}")
                eng = nc.sync if i % 2 == 0 else nc.scalar
                eng.dma_start(
                    kt[:], key_s.rearrange("(c p2 p) h -> c p p2 h", c=4, p2=2)[i])
                ksb.append(kt)

            cv = consts.tile([128, 2, NH + 1], F32)
            for oh in range(2):
                nc.vector.tensor_scalar(
                    cv[:, oh, 0:1], vcol[:, oh : oh + 1], float(D_LIN), None, ALU.mult)
                for n in range(NH):
                    nc.vector.tensor_scalar(
                        cv[:, oh, 1 + n : 2 + n], vcol[:, oh : oh + 1],
                        float(BETAS[n]), None, ALU.mult)

            qT = sb.tile([128, 2, TSH], F32)
            for hh in range(2):
                pt = psA.tile([128, 128], F32, tag="tp")
                nc.tensor.transpose(pt[:], qsb[:, hh, :], ident[:])
                nc.scalar.copy(qT[:, hh, :], pt[:])
            wqT = sb.tile([128, 2, HSZ], F32)
            for oh in range(2):
                for hh in range(2):
                    pt = psA.tile([128, 128], F32, tag="tp")
                    nc.tensor.transpose(pt[:], wq2[:, oh, hh * 128 : (hh + 1) * 128], ident[:])
                    nc.scalar.copy(wqT[:, hh, oh * 128 : (oh + 1) * 128], pt[:])

            Pq = [sb.tile([128, 2, 2, TSH], BF16, tag=f"Pq{n}", name=f"Pq{n}")
                  for n in range(NH + 1)]
            nc.vector.memset(Pq[0][:, :, 0, :], 0.0)
            nc.vector.memset(Pq[0][:, :, 1, :], 1.0)
            for oh in range(2):
                pq = psA.tile([128, TSH], F32, tag="tp")
                for hh in range(2):
                    nc.tensor.matmul(
                        pq[:], wqT[:, hh, oh * 128 : (oh + 1) * 128], qT[:, hh, :],
                        start=(hh == 0), stop=(hh == 1))
                nc.scalar.activation(Pq[1][:, oh, 0, :], pq[:], AF.Sin, bias=zero[:], scale=W0)
                nc.scalar.activation(Pq[1][:, oh, 1, :], pq[:], AF.Sin, bias=halfpi[:], scale=W0)
            Dq = sb.tile([128, 2, 2, TSH], BF16)
            for sc_ in range(2):
                nc.vector.tensor_scalar(
                    Dq[:, :, sc_, :], Pq[1][:, :, 1, :], 2.0, None, ALU.mult)
            for n in range(2, NH + 1):
                t1 = sb.tile([128, 2, 2, TSH], BF16, tag="qtmp", bufs=2, name=f"qt{n}")
                nc.vector.tensor_tensor(t1[:], Dq[:], Pq[n - 1][:], ALU.mult)
                nc.vector.tensor_tensor(Pq[n][:], t1[:], Pq[n - 2][:], ALU.subtract)

            ones_b = consts.tile([128, 2, TSH], BF16)
            nc.vector.memset(ones_b[:], 1.0)
            lh_d = sb.tile([128, 2, TSH], BF16)
            lh_S = [sb.tile([128, 2, TSH], BF16, tag=f"lhS{n}", name=f"lhS{n}")
                    for n in range(NH)]
            lh_C = [sb.tile([128, 2, TSH], BF16, tag=f"lhC{n}", name=f"lhC{n}")
                    for n in range(NH)]
            for oh in range(2):
                nc.scalar.mul(lh_d[:, oh, :], ones_b[:, oh, :], cv[:, oh, 0:1])
                for n in range(NH):
                    if n % 2 == 0:
                        nc.scalar.mul(
                            lh_S[n][:, oh, :], Pq[n + 1][:, oh, 0, :],
                            cv[:, oh, 1 + n : 2 + n])
                        nc.scalar.mul(
                            lh_C[n][:, oh, :], Pq[n + 1][:, oh, 1, :],
                            cv[:, oh, 1 + n : 2 + n])
                    else:
                        nc.vector.tensor_scalar(
                            lh_S[n][:, oh, :], Pq[n + 1][:, oh, 0, :],
                            cv[:, oh, 1 + n : 2 + n], None, ALU.mult)
                        nc.vector.tensor_scalar(
                            lh_C[n][:, oh, :], Pq[n + 1][:, oh, 1, :],
                            cv[:, oh, 1 + n : 2 + n], None, ALU.mult)

            wkT = sb.tile([128, 2, HSZ], F32)
            for oh in range(2):
                for hh in range(2):
                    pt = psA.tile([128, 128], F32, tag="tp")
                    nc.tensor.transpose(pt[:], wk2[:, oh, hh * 128 : (hh + 1) * 128], ident[:])
                    nc.scalar.copy(wkT[:, hh, oh * 128 : (oh + 1) * 128], pt[:])
            kT = sb.tile([128, 2, SRC], F32)
            for blk in range(8):
                src_tile = ksb[blk // 2][:, blk % 2, :]
                for hh in range(2):
                    pt = psA.tile([128, 128], F32, tag="tp")
                    nc.tensor.transpose(pt[:], src_tile[:, hh * 128 : (hh + 1) * 128], ident[:])
                    nc.scalar.copy(kT[:, hh, blk * 128 : (blk + 1) * 128], pt[:])

            Pk = [sb.tile([128, 2, 2, SRC], BF16, tag=f"Pk{n}", name=f"Pk{n}")
                  for n in range(NH + 1)]
            nc.vector.memset(Pk[0][:, :, 0, :], 0.0)
            nc.vector.memset(Pk[0][:, :, 1, :], 1.0)
            kraw = sb.tile([128, 2, SRC], BF16)
            for oh in range(2):
                pk = psB.tile([128, SRC], F32, tag="pk", name=f"pk{oh}")
                for sc in range(2):
                    for hh in range(2):
                        nc.tensor.matmul(
                            pk[:, sc * 512 : (sc + 1) * 512],
                            wkT[:, hh, oh * 128 : (oh + 1) * 128],
                            kT[:, hh, sc * 512 : (sc + 1) * 512],
                            start=(hh == 0), stop=(hh == 1))
                nc.scalar.activation(Pk[1][:, oh, 0, :], pk[:], AF.Sin, bias=zero[:], scale=W0)
                nc.scalar.activation(Pk[1][:, oh, 1, :], pk[:], AF.Sin, bias=halfpi[:], scale=W0)
                nc.scalar.copy(kraw[:, oh, :], pk[:])
            Dk = sb.tile([128, 2, 2, SRC], BF16)
            for sc_ in range(2):
                nc.vector.tensor_scalar(
                    Dk[:, :, sc_, :], Pk[1][:, :, 1, :], 2.0, None, ALU.mult)

            psc = psC.tile([128, SRC], F32)
            sc_started = [False, False]

            def emit_pair(lh, rhs_fn, last=False):
                for sc in range(2):
                    for oh in range(2):
                        is_last = last and sc == 1 and oh == 1
                        nc.tensor.matmul(
                            psc[:, sc * 512 : (sc + 1) * 512],
                            lh[:, oh, :],
                            rhs_fn(oh, sc),
                            start=not sc_started[sc], stop=is_last)
                        sc_started[sc] = True

            emit_pair(lh_d, lambda oh, sc: kraw[:, oh, sc * 512 : (sc + 1) * 512])
            emit_pair(lh_S[0], lambda oh, sc: Pk[1][:, oh, 1, sc * 512 : (sc + 1) * 512])
            emit_pair(lh_C[0], lambda oh, sc: Pk[1][:, oh, 0, sc * 512 : (sc + 1) * 512])
            for n in range(2, NH + 1):
                t1 = sb.tile([128, 2, 2, SRC], BF16, tag="ktmp", bufs=2, name=f"kt{n}")
                nc.vector.tensor_tensor(t1[:], Dk[:], Pk[n - 1][:], ALU.mult)
                nc.vector.tensor_tensor(Pk[n][:], t1[:], Pk[n - 2][:], ALU.subtract)
                emit_pair(
                    lh_S[n - 1],
                    lambda oh, sc, n=n: Pk[n][:, oh, 1, sc * 512 : (sc + 1) * 512])
                emit_pair(
                    lh_C[n - 1],
                    lambda oh, sc, n=n: Pk[n][:, oh, 0, sc * 512 : (sc + 1) * 512],
                    last=(n == NH))

            negmax = sb.tile([128, 1], F32)
            nc.vector.tensor_reduce(
                negmax[:], psc[:], axis=mybir.AxisListType.X, op=ALU.max, negate=True)
            esb = sb.tile([128, SRC], F32)
            nc.scalar.activation(esb[:], psc[:], AF.Exp, bias=negmax[:])
            denom = sb.tile([128, 1], F32)
            nc.vector.tensor_reduce(
                denom[:], esb[:], axis=mybir.AxisListType.X, op=ALU.add)
            rden = sb.tile([128, 1], F32)
            nc.vector.reciprocal(rden[:], denom[:])
            outsb = sb.tile([128, SRC], F32)
            nc.vector.tensor_scalar(outsb[:], esb[:], rden[:], None, ALU.mult)
            nc.sync.dma_start(out[:], outsb[:])

    nc.compile()
    return nc


_NC_CACHE = None


def kernel(**inputs) -> np.ndarray:
    global _NC_CACHE
    query = np.ascontiguousarray(np.asarray(inputs["query"], dtype=np.float32))
    key = np.ascontiguousarray(np.asarray(inputs["key"], dtype=np.float32))
    Wq = np.ascontiguousarray(np.asarray(inputs["Wq"], dtype=np.float32))
    Wk = np.ascontiguousarray(np.asarray(inputs["Wk"], dtype=np.float32))
    v = np.ascontiguousarray(np.asarray(inputs["v"], dtype=np.float32))

    if _NC_CACHE is None:
        _NC_CACHE = _build_nc()
    nc = _NC_CACHE

    in_maps = []
    for c in range(NC):
        b, th = c // 2, c % 2
        in_maps.append({
            "query_s": query[b, th * TSH : (th + 1) * TSH, :],
            "key_s": key[b],
            "wq": Wq,
            "wk": Wk,
            "vv": v,
        })
    res = run_bass_kernel_spmd(nc, in_maps, core_ids=list(range(NC)))
    out = np.empty((BSZ, TGT, SRC), dtype=np.float32)
    for c in range(NC):
        b, th = c // 2, c % 2
        out[b, th * TSH : (th + 1) * TSH, :] = res.results[c]["out"]
    return out


if __name__ == "__main__":
    rng = np.random.default_rng(0)
    ins = {
        "query": rng.standard_normal((BSZ, TGT, HSZ), dtype=np.float32),
        "key": rng.standard_normal((BSZ, SRC, HSZ), dtype=np.float32),
        "Wq": rng.standard_normal((HSZ, HSZ), dtype=np.float32) / 16,
        "Wk": rng.standard_normal((HSZ, HSZ), dtype=np.float32) / 16,
        "v": rng.standard_normal((HSZ,), dtype=np.float32) / 16,
        "v_bias": np.zeros(1, dtype=np.float32),
    }
    o = kernel(**ins)
    print("out", o.shape, o.dtype, o.sum(-1)[:2, :4])


# revision 9
# speedup vs baseline: 1.8929x; 1.0433x over previous
import numpy as np

import concourse.bass as bass
import concourse.mybir as mybir
import concourse.tile as tile
from concourse import bacc
from concourse.bass_utils import run_bass_kernel_spmd
from concourse.masks import make_identity

AF = mybir.ActivationFunctionType
ALU = mybir.AluOpType
F32 = mybir.dt.float32
BF16 = mybir.dt.bfloat16

BSZ, TGT, SRC, HSZ = 4, 256, 1024, 256
TSH = TGT // 2
NC = 8

W0 = 0.55
HARMONICS = [1, 2, 3, 4, 5, 6, 8, 10]
D_LIN = 0.1754
BETAS = [
    0.56254, 0.20271, 0.08184, 0.03607, 0.01283, 0.00855, 0.00268, 0.00032,
]
NH = len(BETAS)
HALFPI = float(np.pi / 2)

CHAIN_STEPS = [
    (2, "D", 1, 0), (3, "D", 2, 1), (4, "D", 3, 2), (5, "D", 4, 3),
    (6, "D", 5, 4), (8, "D2", 6, 4), (10, "D2", 8, 6),
]


def _build_nc():
    nc = bacc.Bacc(None, target_bir_lowering=False)

    query_s = nc.declare_dram_parameter("query_s", [TSH, HSZ], F32, isOutput=False)
    key_s = nc.declare_dram_parameter("key_s", [SRC, HSZ], F32, isOutput=False)
    wq = nc.declare_dram_parameter("wq", [HSZ, HSZ], F32, isOutput=False)
    wk = nc.declare_dram_parameter("wk", [HSZ, HSZ], F32, isOutput=False)
    vv = nc.declare_dram_parameter("vv", [HSZ], F32, isOutput=False)
    out = nc.declare_dram_parameter("out", [TSH, SRC], F32, isOutput=True)

    with tile.TileContext(nc) as tc:
        with (
            tc.tile_pool(name="consts", bufs=1) as consts,
            tc.tile_pool(name="sb", bufs=1) as sb,
            tc.tile_pool(name="psA", bufs=4, space=bass.MemorySpace.PSUM) as psA,
            tc.tile_pool(name="psB", bufs=1, space=bass.MemorySpace.PSUM) as psB,
            tc.tile_pool(name="psC", bufs=1, space=bass.MemorySpace.PSUM) as psC,
        ):
            ident = consts.tile([128, 128], F32)
            make_identity(nc, ident[:])
            halfpi = consts.tile([128, 1], F32)
            nc.vector.memset(halfpi[:], HALFPI)
            zero = consts.tile([128, 1], F32)
            nc.vector.memset(zero[:], 0.0)

            qsb = sb.tile([128, 2, 128], F32)
            nc.sync.dma_start(qsb[:], query_s.rearrange("t (b h) -> t b h", h=128))
            wq2 = sb.tile([128, 2, HSZ], F32)
            nc.scalar.dma_start(wq2[:], wq.rearrange("(a p) h -> p a h", p=128))
            wk2 = sb.tile([128, 2, HSZ], F32)
            nc.gpsimd.dma_start(wk2[:], wk.rearrange("(a p) h -> p a h", p=128))
            vcol = consts.tile([128, 2], F32)
            nc.gpsimd.dma_start(vcol[:], vv.rearrange("(a p) -> p a", p=128))
            ksb = []
            for i in range(4):
                kt = sb.tile([128, 2, HSZ], F32, tag=f"ksb{i}", name=f"ksb{i}")
                eng = nc.sync if i % 2 == 0 else nc.scalar
                eng.dma_start(
                    kt[:], key_s.rearrange("(c p2 p) h -> c p p2 h", c=4, p2=2)[i])
                ksb.append(kt)

            cv = consts.tile([128, 2, NH + 1], F32)
            for oh in range(2):
                nc.vector.tensor_scalar(
                    cv[:, oh, 0:1], vcol[:, oh : oh + 1], float(D_LIN), None, ALU.mult)
                for n in range(NH):
                    nc.vector.tensor_scalar(
                        cv[:, oh, 1 + n : 2 + n], vcol[:, oh : oh + 1],
                        float(BETAS[n]), None, ALU.mult)

            qT = sb.tile([128, 2, TSH], F32)
            for hh in range(2):
                pt = psA.tile([128, 128], F32, tag="tp")
                nc.tensor.transpose(pt[:], qsb[:, hh, :], ident[:])
                nc.scalar.copy(qT[:, hh, :], pt[:])
            wqT = sb.tile([128, 2, HSZ], F32)
            for oh in range(2):
                for hh in range(2):
                    pt = psA.tile([128, 128], F32, tag="tp")
                    nc.tensor.transpose(pt[:], wq2[:, oh, hh * 128 : (hh + 1) * 128], ident[:])
                    nc.scalar.copy(wqT[:, hh, oh * 128 : (oh + 1) * 128], pt[:])

            qmems = sorted({0, 1} | {m for st in CHAIN_STEPS for m in (st[0], st[2], st[3])})
            Pq = {m: sb.tile([128, 2, 2, TSH], BF16, tag=f"Pq{m}", name=f"Pq{m}")
                  for m in qmems}
            nc.gpsimd.memset(Pq[0][:, :, 0, :], 0.0)
            nc.gpsimd.memset(Pq[0][:, :, 1, :], 1.0)
            for oh in range(2):
                pq = psA.tile([128, TSH], F32, tag="tp")
                for hh in range(2):
                    nc.tensor.matmul(
                        pq[:], wqT[:, hh, oh * 128 : (oh + 1) * 128], qT[:, hh, :],
                        start=(hh == 0), stop=(hh == 1))
                nc.scalar.activation(Pq[1][:, oh, 0, :], pq[:], AF.Sin, bias=zero[:], scale=W0)
                nc.scalar.activation(Pq[1][:, oh, 1, :], pq[:], AF.Sin, bias=halfpi[:], scale=W0)
            Dq = sb.tile([128, 2, 2, TSH], BF16)
            Dq2 = sb.tile([128, 2, 2, TSH], BF16)
            for sc_ in range(2):
                nc.vector.tensor_scalar(
                    Dq[:, :, sc_, :], Pq[1][:, :, 1, :], 2.0, None, ALU.mult)
            for (m, mk, m1, m2) in CHAIN_STEPS:
                mult = Dq if mk == "D" else Dq2
                t1 = sb.tile([128, 2, 2, TSH], BF16, tag="qtmp", bufs=2, name=f"qt{m}")
                nc.vector.tensor_tensor(t1[:], mult[:], Pq[m1][:], ALU.mult)
                nc.vector.tensor_tensor(Pq[m][:], t1[:], Pq[m2][:], ALU.subtract)
                if m == 2:
                    for sc_ in range(2):
                        nc.vector.tensor_scalar(
                            Dq2[:, :, sc_, :], Pq[2][:, :, 1, :], 2.0, None, ALU.mult)

            ones_b = consts.tile([128, 2, TSH], BF16)
            nc.gpsimd.memset(ones_b[:], 1.0)
            lh_d = sb.tile([128, 2, TSH], BF16)
            lh_S = [sb.tile([128, 2, TSH], BF16, tag=f"lhS{n}", name=f"lhS{n}")
                    for n in range(NH)]
            lh_C = [sb.tile([128, 2, TSH], BF16, tag=f"lhC{n}", name=f"lhC{n}")
                    for n in range(NH)]
            for oh in range(2):
                nc.scalar.mul(lh_d[:, oh, :], ones_b[:, oh, :], cv[:, oh, 0:1])
                for j, m in enumerate(HARMONICS):
                    if j % 2 == 0:
                        nc.scalar.mul(
                            lh_S[j][:, oh, :], Pq[m][:, oh, 0, :],
                            cv[:, oh, 1 + j : 2 + j])
                        nc.scalar.mul(
                            lh_C[j][:, oh, :], Pq[m][:, oh, 1, :],
                            cv[:, oh, 1 + j : 2 + j])
                    else:
                        nc.vector.tensor_scalar(
                            lh_S[j][:, oh, :], Pq[m][:, oh, 0, :],
                            cv[:, oh, 1 + j : 2 + j], None, ALU.mult)
                        nc.vector.tensor_scalar(
                            lh_C[j][:, oh, :], Pq[m][:, oh, 1, :],
                            cv[:, oh, 1 + j : 2 + j], None, ALU.mult)

            wkT = sb.tile([128, 2, HSZ], F32)
            for oh in range(2):
                for hh in range(2):
                    pt = psA.tile([128, 128], F32, tag="tp")
                    nc.tensor.transpose(pt[:], wk2[:, oh, hh * 128 : (hh + 1) * 128], ident[:])
                    nc.scalar.copy(wkT[:, hh, oh * 128 : (oh + 1) * 128], pt[:])
            kT = sb.tile([128, 2, SRC], F32)
            for blk in range(8):
                src_tile = ksb[blk // 2][:, blk % 2, :]
                for hh in range(2):
                    pt = psA.tile([128, 128], F32, tag="tp")
                    nc.tensor.transpose(pt[:], src_tile[:, hh * 128 : (hh + 1) * 128], ident[:])
                    nc.scalar.copy(kT[:, hh, blk * 128 : (blk + 1) * 128], pt[:])

            Pk = {m: sb.tile([128, 2, 2, SRC], BF16, tag=f"Pk{m}", name=f"Pk{m}")
                  for m in qmems}
            nc.gpsimd.memset(Pk[0][:, :, 0, :], 0.0)
            nc.gpsimd.memset(Pk[0][:, :, 1, :], 1.0)
            kraw = sb.tile([128, 2, SRC], BF16)
            for oh in range(2):
                pk = psB.tile([128, SRC], F32, tag="pk", name=f"pk{oh}")
                for sc in range(2):
                    for hh in range(2):
                        nc.tensor.matmul(
                            pk[:, sc * 512 : (sc + 1) * 512],
                            wkT[:, hh, oh * 128 : (oh + 1) * 128],
                            kT[:, hh, sc * 512 : (sc + 1) * 512],
                            start=(hh == 0), stop=(hh == 1))
                nc.scalar.activation(Pk[1][:, oh, 0, :], pk[:], AF.Sin, bias=zero[:], scale=W0)
                nc.scalar.activation(Pk[1][:, oh, 1, :], pk[:], AF.Sin, bias=halfpi[:], scale=W0)
                nc.scalar.copy(kraw[:, oh, :], pk[:])
            Dk = sb.tile([128, 2, 2, SRC], BF16)
            Dk2 = sb.tile([128, 2, 2, SRC], BF16)
            for sc_ in range(2):
                nc.vector.tensor_scalar(
                    Dk[:, :, sc_, :], Pk[1][:, :, 1, :], 2.0, None, ALU.mult)

            psc = psC.tile([128, SRC], F32)
            sc_started = [False, False]

            def emit_pair(lh, rhs_fn, last=False):
                for sc in range(2):
                    for oh in range(2):
                        is_last = last and sc == 1 and oh == 1
                        nc.tensor.matmul(
                            psc[:, sc * 512 : (sc + 1) * 512],
                            lh[:, oh, :],
                            rhs_fn(oh, sc),
                            start=not sc_started[sc], stop=is_last)
                        sc_started[sc] = True

            emit_pair(lh_d, lambda oh, sc: kraw[:, oh, sc * 512 : (sc + 1) * 512])
            emit_pair(lh_S[0], lambda oh, sc: Pk[1][:, oh, 1, sc * 512 : (sc + 1) * 512])
            emit_pair(lh_C[0], lambda oh, sc: Pk[1][:, oh, 0, sc * 512 : (sc + 1) * 512])
            for (m, mk, m1, m2) in CHAIN_STEPS:
                mult = Dk if mk == "D" else Dk2
                t1 = sb.tile([128, 2, 2, SRC], BF16, tag="ktmp", bufs=2, name=f"kt{m}")
                nc.vector.tensor_tensor(t1[:], mult[:], Pk[m1][:], ALU.mult)
                nc.vector.tensor_tensor(Pk[m][:], t1[:], Pk[m2][:], ALU.subtract)
                if m == 2:
                    for sc_ in range(2):
                        nc.vector.tensor_scalar(
                            Dk2[:, :, sc_, :], Pk[2][:, :, 1, :], 2.0, None, ALU.mult)
                j = HARMONICS.index(m)
                emit_pair(
                    lh_S[j],
                    lambda oh, sc, m=m: Pk[m][:, oh, 1, sc * 512 : (sc + 1) * 512])
                emit_pair(
                    lh_C[j],
                    lambda oh, sc, m=m: Pk[m][:, oh, 0, sc * 512 : (sc + 1) * 512],
                    last=(m == CHAIN_STEPS[-1][0]))

            esb = sb.tile([128, SRC], F32)
            nc.scalar.activation(esb[:], psc[:], AF.Exp, bias=zero[:])
            denom = sb.tile([128, 1], F32)
            nc.vector.tensor_reduce(
                denom[:], esb[:], axis=mybir.AxisListType.X, op=ALU.add)
            rden = sb.tile([128, 1], F32)
            nc.vector.reciprocal(rden[:], denom[:])
            outsb = sb.tile([128, SRC], F32)
            nc.vector.tensor_scalar(outsb[:], esb[:], rden[:], None, ALU.mult)
            nc.sync.dma_start(out[:], outsb[:])

    nc.compile()
    return nc


_NC_CACHE = None


def kernel(**inputs) -> np.ndarray:
    global _NC_CACHE
    query = np.ascontiguousarray(np.asarray(inputs["query"], dtype=np.float32))
    key = np.ascontiguousarray(np.asarray(inputs["key"], dtype=np.float32))
    Wq = np.ascontiguousarray(np.asarray(inputs["Wq"], dtype=np.float32))
    Wk = np.ascontiguousarray(np.asarray(inputs["Wk"], dtype=np.float32))
    v = np.ascontiguousarray(np.asarray(inputs["v"], dtype=np.float32))

    if _NC_CACHE is None:
        _NC_CACHE = _build_nc()
    nc = _NC_CACHE

    in_maps = []
    for c in range(NC):
        b, th = c // 2, c % 2
        in_maps.append({
            "query_s": query[b, th * TSH : (th + 1) * TSH, :],
            "key_s": key[b],
            "wq": Wq,
            "wk": Wk,
            "vv": v,
        })
    res = run_bass_kernel_spmd(nc, in_maps, core_ids=list(range(NC)))
    out = np.empty((BSZ, TGT, SRC), dtype=np.float32)
    for c in range(NC):
        b, th = c // 2, c % 2
        out[b, th * TSH : (th + 1) * TSH, :] = res.results[c]["out"]
    return out


if __name__ == "__main__":
    rng = np.random.default_rng(0)
    ins = {
        "query": rng.standard_normal((BSZ, TGT, HSZ), dtype=np.float32),
        "key": rng.standard_normal((BSZ, SRC, HSZ), dtype=np.float32),
        "Wq": rng.standard_normal((HSZ, HSZ), dtype=np.float32) / 16,
        "Wk": rng.standard_normal((HSZ, HSZ), dtype=np.float32) / 16,
        "v": rng.standard_normal((HSZ,), dtype=np.float32) / 16,
        "v_bias": np.zeros(1, dtype=np.float32),
    }
    o = kernel(**ins)
    print("out", o.shape, o.dtype, o.sum(-1)[:2, :4])
